# revision 17
# baseline (speedup 1.0000x reference)
"""Causal multi-head attention (B=4, N=2048, D=1024, H=16, dk=dv=64) on 8 Trainium2
NeuronCores.

Sharding: tensor-parallel over heads — core c computes QKV projections and
attention for heads 2c, 2c+1 over the full sequence, then an AllToAll exchanges
attention outputs so each core computes the output projection for a strided
1/8 slice of tokens (positions [256c, 256c+256) of every batch).

All matmul operands are bf16 (fp32 PSUM accumulation): one-pass PE streams and
fast weight loads, vs fp32r's two-pass mode that measured ~3x slower. The
projection and attention loops are interleaved per batch so the tensor engine's
projection work for supertile tt+1 overlaps the scalar engine's softmax-exp
backlog from attention block qq=tt.

Attention uses the S^T layout (keys on partitions, queries on free): scores for
both heads land in one [128, 1024] PSUM tile (head1 via tile_position=(64,0)),
one Exp activation covers both heads through a strided [128, 2, L] view, and
P@V consumes P^T directly. Causal structure: only live query columns
[128r, 512) are computed/exp'd/streamed for diagonal-band key tiles, and only
the single triangular 128x128 sub-tile per band tile gets the mask multiply.
V is transposed on the PE into key-major [V_h0 | 1 | V_h1 | 1] blocks; the ones
column accumulates softmax denominators during P@V; normalization is a
reciprocal + partition-broadcast multiply after each query block.
"""

import numpy as np

B, N, D = 4, 2048, 1024
H, DK = 16, 64
NCORES = 8
TOK = B * N                 # 8192 tokens
KT = D // 128               # 8 contraction tiles of d_model
TPC = TOK // NCORES         # 1024 tokens per core in the output projection
NTT = N // 512              # 4 supertiles of 512 tokens per batch

_CACHE = {}
TRACE = False
LAST_EXEC_NS = None
LAST_RESULTS = None


def _build():
    import concourse.tile as tile
    from concourse import bacc, mybir

    F32 = mybir.dt.float32
    BF16 = mybir.dt.bfloat16
    Exp = mybir.ActivationFunctionType.Exp
    mult = mybir.AluOpType.mult

    nc = bacc.Bacc("TRN2", target_bir_lowering=False, debug=False, num_devices=NCORES)

    xT_d = nc.dram_tensor("xT", [D, TOK], BF16, kind="ExternalInput")
    wq_d = nc.dram_tensor("wq", [128, KT * 128], BF16, kind="ExternalInput")
    wk_d = nc.dram_tensor("wk", [128, KT * 128], BF16, kind="ExternalInput")
    wv_d = nc.dram_tensor("wv", [128, KT * 128], BF16, kind="ExternalInput")
    bq_d = nc.dram_tensor("bq", [128, 1], F32, kind="ExternalInput")
    bk_d = nc.dram_tensor("bk", [128, 1], F32, kind="ExternalInput")
    bv_d = nc.dram_tensor("bv", [128, 1], F32, kind="ExternalInput")
    wo_d = nc.dram_tensor("wo", [128, KT * D], BF16, kind="ExternalInput")
    mask2_d = nc.dram_tensor("mask2", [128, 256], BF16, kind="ExternalInput")
    ident_d = nc.dram_tensor("ident", [128, 128], BF16, kind="ExternalInput")
    out_d = nc.dram_tensor("out", [TPC, D], F32, kind="ExternalOutput")

    with tile.TileContext(nc) as tc:
        with tc.tile_pool(name="dram", bufs=1, space="DRAM") as dram:
            ot_dram = [
                dram.tile([NCORES, 128, 256], BF16, name=f"ot_dram{b}")
                for b in range(B)
            ]
            a2a_out = [
                dram.tile([NCORES, 128, 256], BF16, name=f"a2a_out{b}")
                for b in range(B)
            ]

            with tc.tile_pool(name="big", bufs=1) as big:
                qt = [big.tile([128, N], BF16, name=f"qt{b}") for b in range(B)]
                kt = [big.tile([128, N], BF16, name=f"kt{b}") for b in range(B)]
                vsb = [big.tile([128, 16 * 130], BF16, name=f"vsb{b}") for b in range(B)]
                oth = [big.tile([128, N], BF16, name=f"oth{b}") for b in range(B)]
                wq_s = big.tile([128, KT * 128], BF16, name="wq_s")
                wk_s = big.tile([128, KT * 128], BF16, name="wk_s")
                wv_s = big.tile([128, KT * 128], BF16, name="wv_s")
                wo_s = big.tile([128, KT * D], BF16, name="wo_s")
                bq_s = big.tile([128, 1], F32, name="bq_s")
                bk_s = big.tile([128, 1], F32, name="bk_s")
                bv_s = big.tile([128, 1], F32, name="bv_s")
                ident = big.tile([128, 128], BF16, name="ident")
                mask2 = big.tile([128, 256], BF16, name="mask2")
                warm = big.tile([128, 1], F32, name="warm")

                nc.sync.dma_start(wq_s[:], wq_d[:])
                nc.sync.dma_start(wk_s[:], wk_d[:])
                nc.sync.dma_start(wv_s[:], wv_d[:])
                nc.sync.dma_start(bq_s[:], bq_d[:])
                nc.sync.dma_start(bk_s[:], bk_d[:])
                nc.sync.dma_start(bv_s[:], bv_d[:])
                nc.sync.dma_start(ident[:], ident_d[:])
                nc.sync.dma_start(mask2[:], mask2_d[:])
                nc.sync.dma_start(wo_s[:], wo_d[:])
                # dummy exp so the ~2.7us activation-table load happens during
                # the startup DMA phase, not on the first attention block
                nc.scalar.activation(warm[:], bq_s[:], Exp, scale=0.0)

                with (
                    tc.tile_pool(name="xt", bufs=18) as xpool,
                    tc.tile_pool(name="vt", bufs=2) as vtpool,
                    tc.tile_pool(name="pt", bufs=4) as ptp,
                    tc.tile_pool(name="rc", bufs=2) as rcp,
                    tc.tile_pool(name="bc", bufs=2) as bcp,
                    tc.tile_pool(name="ot3", bufs=2) as ot3,
                    tc.tile_pool(name="os3", bufs=2) as os3,
                    tc.tile_pool(name="pps", bufs=2, space="PSUM") as pps,
                    tc.tile_pool(name="sps", bufs=2, space="PSUM") as sps,
                    tc.tile_pool(name="ops", bufs=1, space="PSUM") as ops,
                ):
                    def _outproj(ob):
                        # out rows [256*ob, 256*(ob+1)) = this core's slice of
                        # batch ob; PSUM comes from the s_ps tag (free between
                        # attention blocks).
                        ot_t = ot3.tile([128, KT * 256], BF16, name="ot_t")
                        for kk in range(KT):
                            nc.sync.dma_start(
                                ot_t[:, 256 * kk:256 * (kk + 1)], a2a_out[ob][kk]
                            )
                        for j in range(2):
                            out_ps0 = sps.tile([128, 512], F32, name="s_ps")
                            out_ps1 = sps.tile([128, 512], F32, name="s_ps")
                            for kk in range(KT):
                                f, l = kk == 0, kk == KT - 1
                                lhs = ot_t[:, 256 * kk + 128 * j:256 * kk + 128 * (j + 1)]
                                nc.tensor.matmul(
                                    out_ps0[:], lhs, wo_s[:, kk * D:kk * D + 512],
                                    start=f, stop=l,
                                )
                                nc.tensor.matmul(
                                    out_ps1[:], lhs, wo_s[:, kk * D + 512:kk * D + 1024],
                                    start=f, stop=l,
                                )
                            out_sb = os3.tile([128, D], F32, name="out_sb")
                            nc.vector.tensor_copy(out_sb[:, 0:512], out_ps0[:])
                            nc.vector.tensor_copy(out_sb[:, 512:1024], out_ps1[:])
                            nc.sync.dma_start(
                                out_d[256 * ob + 128 * j:256 * ob + 128 * (j + 1), :],
                                out_sb[:],
                            )

                    def chain_gen(b, tt):
                        """QKV chain matmuls + evacs for supertile (b, tt),
                        yielding between instructions so they can fill the
                        tensor engine's exp-wait gaps inside the previous
                        attention block. V transposes are NOT here (they use
                        the s_ps PSUM tag, which attention scores own)."""
                        gsl = slice(N * b + 512 * tt, N * b + 512 * (tt + 1))
                        lsl = slice(512 * tt, 512 * (tt + 1))
                        xts = []
                        for kk in range(KT):
                            xt = xpool.tile([128, 512], BF16, name="xt")
                            nc.gpsimd.dma_start(
                                xt[:], xT_d[128 * kk:128 * (kk + 1), gsl]
                            )
                            xts.append(xt)
                        for w_s, b_s, dst in (
                            (wq_s, bq_s, qt[b]),
                            (wk_s, bk_s, kt[b]),
                        ):
                            acc = pps.tile([128, 512], F32, name="acc")
                            for kk in range(KT):
                                nc.tensor.matmul(
                                    acc[:], w_s[:, 128 * kk:128 * (kk + 1)],
                                    xts[kk][:], start=kk == 0, stop=kk == KT - 1,
                                )
                                yield
                            nc.vector.tensor_scalar_add(dst[:, lsl], acc[:], b_s[:])
                            yield
                        acc = pps.tile([128, 512], F32, name="acc")
                        for kk in range(KT):
                            nc.tensor.matmul(
                                acc[:], wv_s[:, 128 * kk:128 * (kk + 1)],
                                xts[kk][:], start=kk == 0, stop=kk == KT - 1,
                            )
                            yield
                        vt_sb = vtpool.tile([128, 512], BF16, name="vt_sb")
                        nc.vector.tensor_scalar_add(vt_sb[:], acc[:], bv_s[:])
                        # hand vt_sb back for the transpose phase
                        _vt_box[(b, tt)] = vt_sb

                    _vt_box = {}
                    _DONE = object()

                    def drive(gen, n):
                        if gen is None:
                            return
                        for _ in range(n):
                            if next(gen, _DONE) is _DONE:
                                return

                    def drain(gen):
                        if gen is None:
                            return
                        for _ in gen:
                            pass

                    def vtrans_phase(b, tt):
                        """V transposes into key-major vsb blocks (s_ps PSUM
                        tag; runs between attention blocks)."""
                        vt_sb = _vt_box.pop((b, tt))
                        for j in range(4):
                            vtr = sps.tile([128, 128], BF16, name="s_ps")
                            nc.tensor.transpose(
                                vtr[:], vt_sb[:, 128 * j:128 * (j + 1)], ident[:]
                            )
                            base = (4 * tt + j) * 130
                            nc.vector.tensor_copy(
                                vsb[b][:, base:base + 130]
                                .rearrange("p (h c) -> p h c", c=65)[:, :, 0:64],
                                vtr[:].rearrange("p (h c) -> p h c", h=2),
                            )
                        nc.vector.memset(
                            vsb[b][:, 130 * 4 * tt:130 * 4 * (tt + 1)]
                            .rearrange("p (n c) -> p n c", c=65)[:, :, 64:65],
                            1.0,
                        )

                    g0 = chain_gen(0, 0)
                    drain(g0)
                    vtrans_phase(0, 0)
                    for b in range(B):
                        for tt in range(NTT):
                            # chain matmuls of the NEXT supertile interleave
                            # into this attention block as PE filler
                            if tt < NTT - 1:
                                nxt, filler = (b, tt + 1), None
                            elif b < B - 1:
                                nxt, filler = (b + 1, 0), None
                            else:
                                nxt = None
                            filler = chain_gen(*nxt) if nxt else None

                            # ---------- attention, query block qq = tt ----------
                            qq = tt
                            kmax = 4 * qq + 3
                            budget = 27 // (kmax + 1) + 1
                            o_ps0 = ops.tile([65, 512], F32, name="o_ps0")
                            o_ps1 = ops.tile([65, 512], F32, name="o_ps1")

                            def pv(p_tile, pkk, last):
                                r = pkk - 4 * qq
                                liv = 128 * r if r > 0 else 0
                                vb = 130 * pkk
                                nc.tensor.matmul(
                                    o_ps0[:, liv:512], vsb[b][:, vb:vb + 65],
                                    p_tile[:, liv:512],
                                    start=pkk == 0, stop=last,
                                )
                                nc.tensor.matmul(
                                    o_ps1[:, liv:512], vsb[b][:, vb + 65:vb + 130],
                                    p_tile[:, 512 + liv:1024],
                                    start=pkk == 0, stop=last,
                                )

                            pend = None
                            for kk in range(kmax + 1):
                                r = kk - 4 * qq
                                liv = 128 * r if r > 0 else 0
                                ksl = slice(128 * kk, 128 * (kk + 1))
                                qsl = slice(512 * qq + liv, 512 * (qq + 1))
                                s_ps = sps.tile([128, 1024], F32, name="s_ps")
                                nc.tensor.matmul(
                                    s_ps[:, liv:512], kt[b][0:64, ksl],
                                    qt[b][0:64, qsl], start=True, stop=True,
                                )
                                nc.tensor.matmul(
                                    s_ps[:, 512 + liv:1024], kt[b][64:128, ksl],
                                    qt[b][64:128, qsl], start=True, stop=True,
                                    tile_position=(64, 0),
                                )
                                p = ptp.tile([128, 1024], BF16, name="p")
                                nc.scalar.activation(
                                    p[:].rearrange("p (h c) -> p h c", c=512)[:, :, liv:512],
                                    s_ps[:].rearrange("p (h c) -> p h c", c=512)[:, :, liv:512],
                                    Exp, scale=0.125,
                                )
                                if r >= 0:
                                    pd = p[:].rearrange("p (h c) -> p h c", c=512)[:, :, liv:liv + 128]
                                    nc.vector.tensor_tensor(
                                        pd, pd,
                                        mask2[:].rearrange("p (h c) -> p h c", c=128),
                                        op=mult,
                                    )
                                if pend is not None:
                                    pv(*pend, last=False)
                                pend = (p, kk)
                                drive(filler, budget)
                            pv(*pend, last=True)
                            drain(filler)

                            for h, o_ps in ((0, o_ps0), (1, o_ps1)):
                                # custom-DVE reciprocal can't read PSUM; stage
                                # the denominator row to SBUF on ScalarE first
                                rc_in = rcp.tile([1, 512], F32, name="rc_in")
                                nc.scalar.copy(rc_in[:], o_ps[64:65, :])
                                rc = rcp.tile([1, 512], F32, name="rc")
                                nc.vector.reciprocal_approx_fast(rc[:], rc_in[:])
                                bc = bcp.tile([64, 512], F32, name="bc", padded_shape=[128, 512])
                                nc.gpsimd.partition_broadcast(bc[:], rc[0:1, :])
                                nc.vector.tensor_tensor(
                                    oth[b][64 * h:64 * (h + 1), 512 * qq:512 * (qq + 1)],
                                    o_ps[0:64, :], bc[:], op=mult,
                                )
                            if nxt:
                                vtrans_phase(*nxt)
                        # output projection for batch b-1 BEFORE this batch's
                        # collective is emitted: all collectives share one
                        # logical processor in Tile's vector clock, so emitting
                        # it after collective(b) makes its a2a_out[b-1] wait
                        # round up to collective(b)'s completion.
                        if b >= 1:
                            _outproj(b - 1)
                        # stage this batch's O^T out to DRAM + AllToAll it
                        for j in range(NCORES):
                            nc.sync.dma_start(
                                ot_dram[b][j], oth[b][:, 256 * j:256 * (j + 1)]
                            )
                        nc.gpsimd.collective_compute(
                            "AllToAll",
                            mybir.AluOpType.bypass,
                            replica_groups=[list(range(NCORES))],
                            ins=[ot_dram[b][:]],
                            outs=[a2a_out[b][:]],
                        )
                    _outproj(B - 1)

    nc.compile()
    return nc


def _host_prep(inputs):
    import ml_dtypes

    bf16 = ml_dtypes.bfloat16
    x = np.asarray(inputs["x"], np.float32)
    Wq = np.asarray(inputs["Wq"], np.float32)
    bq = np.asarray(inputs["bq"], np.float32)
    Wk = np.asarray(inputs["Wk"], np.float32)
    bk = np.asarray(inputs["bk"], np.float32)
    Wv = np.asarray(inputs["Wv"], np.float32)
    bv = np.asarray(inputs["bv"], np.float32)
    Wo = np.asarray(inputs["Wo"], np.float32)

    xT = np.ascontiguousarray(x.reshape(TOK, D).T).astype(bf16)
    # wo_prep[p, 1024*kk + i] = Wo[i, 128*kk + p]
    wo_prep = np.ascontiguousarray(
        Wo.T.reshape(KT, 128, D).transpose(1, 0, 2).reshape(128, KT * D)
    ).astype(bf16)
    ident = np.eye(128, dtype=np.float32).astype(bf16)
    k_idx = np.arange(128)[:, None]
    q_idx = np.arange(128)[None, :]
    tri = (q_idx >= k_idx).astype(np.float32)
    mask2 = np.concatenate([tri, tri], axis=1).astype(bf16)

    in_maps = []
    for c in range(NCORES):
        sl = slice(128 * c, 128 * (c + 1))
        # w_prep[p, 128*kk + f] = W[128c + f, 128*kk + p]
        def prep_w(W):
            wT = W[sl].T  # [1024, 128]
            return np.ascontiguousarray(
                wT.reshape(KT, 128, 128).transpose(1, 0, 2).reshape(128, KT * 128)
            ).astype(bf16)

        in_maps.append({
            "xT": xT,
            "wq": prep_w(Wq),
            "wk": prep_w(Wk),
            "wv": prep_w(Wv),
            "bq": np.ascontiguousarray(bq[sl].reshape(128, 1)),
            "bk": np.ascontiguousarray(bk[sl].reshape(128, 1)),
            "bv": np.ascontiguousarray(bv[sl].reshape(128, 1)),
            "wo": wo_prep,
            "mask2": mask2,
            "ident": ident,
        })
    return in_maps


def kernel(**inputs):
    global LAST_EXEC_NS, LAST_RESULTS
    from concourse.bass_utils import run_bass_kernel_spmd

    if "nc" not in _CACHE:
        _CACHE["nc"] = _build()
    nc = _CACHE["nc"]
    in_maps = _host_prep(inputs)
    res = run_bass_kernel_spmd(nc, in_maps, list(range(NCORES)), trace=TRACE)
    LAST_EXEC_NS = res.exec_time_ns
    LAST_RESULTS = res
    # core c owns positions [256c, 256c+256) of every batch, rows ordered (b, i)
    out = np.empty((B, N, D), np.float32)
    for c in range(NCORES):
        oc = np.asarray(res.results[c]["out"], np.float32).reshape(B, 256, D)
        out[:, 256 * c:256 * (c + 1), :] = oc
    return out
